# revision 3
# baseline (speedup 1.0000x reference)
"""Multi-head attention (B=4, N=M=2048, D=1024, H=16) on 8 trn2 NeuronCores.

Sharding: core c = (batch b = c//2, head-group hg = c%2 of 8 heads).
Each core computes its 8 heads end-to-end (fc_Q/K/V column-sharded by head,
fc_O row-sharded); the fc_O all-reduce over the 2 cores of a batch is done
in the host-side gather (a single np add), along with the +bo bias.

Host-side prep (sharding/relayout only):
  - Q/K/V are pre-transposed (feature-major) so every matmul contracts over
    the partition dim without any on-chip transposes.
  - Key rows with mask=True contribute exactly 0 to softmax (exp(-inf)); they
    are compacted away host-side and the key dim padded to a multiple of 128
    with bias^T rows of -1e30 (exp -> 0). ~Halves all key-side work.
  - bias enters via exp(s*scale + b) = exp(s*scale) * exp(b): exp(bias^T) is
    computed once on-chip (ACT) and multiplied in on DVE at bf16 2x rate.
  - softmax denominator comes free from a ones-column appended to V; the
    1/denom normalization is folded into the PSUM->SBUF copyback multiply
    before the output projection.
"""

import os
from contextlib import ExitStack

import numpy as np
import ml_dtypes

import concourse.bass as bass
import concourse.tile as tile
from concourse import bacc, mybir
from concourse.bass_utils import run_bass_kernel_spmd

BF16 = mybir.dt.bfloat16
F32 = mybir.dt.float32
AF = mybir.ActivationFunctionType

B, N, M, D, H = 4, 2048, 2048, 1024, 16
HG = 2            # head-groups (cores per batch)
HL = H // HG      # heads per core
HD = D // H       # head dim
DG = HL * HD      # per-core projection width (512)
P = 128
DC = D // P       # D chunks (8)
DCA = DC + 1      # aug chunks for V (bias row)
DGC = DG // P     # head-dim chunks per core (4)
NI5 = N // 512    # query tiles of 512
NEG = -1e30
SCALE = 1.0 / float(np.sqrt(HD))

_cache: dict[int, object] = {}


def _build(m_pad: int):
    NJ = m_pad // P
    nc = bacc.Bacc("TRN2", target_bir_lowering=False, debug=False, num_devices=8)

    qt_d = nc.dram_tensor("qt", [D, N], BF16, kind="ExternalInput").ap()
    kt_d = nc.dram_tensor("kt", [D, m_pad], BF16, kind="ExternalInput").ap()
    vt_d = nc.dram_tensor("vt", [DCA * P, m_pad], BF16, kind="ExternalInput").ap()
    bt_d = nc.dram_tensor("bt", [m_pad, N], F32, kind="ExternalInput").ap()
    wqt_d = nc.dram_tensor("wqt", [D, DG], BF16, kind="ExternalInput").ap()
    wkt_d = nc.dram_tensor("wkt", [D, DG], BF16, kind="ExternalInput").ap()
    wvt_d = nc.dram_tensor("wvt", [DCA * P, DG], BF16, kind="ExternalInput").ap()
    wot_d = nc.dram_tensor("wot", [DG, D], BF16, kind="ExternalInput").ap()
    bq_d = nc.dram_tensor("bqv", [P, DGC], F32, kind="ExternalInput").ap()
    bk_d = nc.dram_tensor("bkv", [P, DGC], F32, kind="ExternalInput").ap()
    out_d = nc.dram_tensor("out", [N, D], F32, kind="ExternalOutput").ap()

    with tile.TileContext(nc) as tc, ExitStack() as ctx:
        singles = ctx.enter_context(tc.tile_pool(name="singles", bufs=1))

        # resident weights
        wq_sb = singles.tile([P, DC, DG], BF16, name="wq")
        nc.sync.dma_start(out=wq_sb, in_=wqt_d.rearrange("(c p) g -> p c g", p=P))
        wk_sb = singles.tile([P, DC, DG], BF16, name="wk")
        nc.sync.dma_start(out=wk_sb, in_=wkt_d.rearrange("(c p) g -> p c g", p=P))
        wv_sb = singles.tile([P, DCA, DG], BF16, name="wv")
        nc.sync.dma_start(out=wv_sb, in_=wvt_d.rearrange("(c p) g -> p c g", p=P))
        wo_sb = singles.tile([P, DGC, D], BF16, name="wo")
        nc.sync.dma_start(out=wo_sb, in_=wot_d.rearrange("(c p) o -> p c o", p=P))
        bq_sb = singles.tile([P, DGC], F32, name="bq")
        nc.sync.dma_start(out=bq_sb, in_=bq_d)
        bk_sb = singles.tile([P, DGC], F32, name="bk")
        nc.sync.dma_start(out=bk_sb, in_=bk_d)

        # persistent activations
        qT = [singles.tile([P, N], BF16, name=f"qT{c}") for c in range(DGC)]
        kT = [singles.tile([P, m_pad], BF16, name=f"kT{c}") for c in range(DGC)]
        v_sb = [singles.tile([P, HL, HD + 1], BF16, name=f"v{j}") for j in range(NJ)]
        eb = [singles.tile([P, N], BF16, name=f"eb{j}") for j in range(NJ)]
        otn = {}
        for hp in range(HL // 2):
            for i in range(NI5):
                otn[(hp, i)] = singles.tile([P, 512], BF16, name=f"otn{hp}_{i}")

        # ---- stage A: projections (inputs streamed via scoped pools) ----
        with ExitStack() as actx:
            ins_pool = actx.enter_context(tc.tile_pool(name="ins", bufs=1))
            psA = actx.enter_context(tc.tile_pool(name="psA", bufs=2, space="PSUM"))

            qt_in = ins_pool.tile([P, DC, N], BF16, name="qt_in")
            for c in range(DC):
                nc.sync.dma_start(out=qt_in[:, c, :], in_=qt_d[c * P:(c + 1) * P, :])
            kt_in = ins_pool.tile([P, DC, m_pad], BF16, name="kt_in")
            for c in range(DC):
                nc.sync.dma_start(out=kt_in[:, c, :], in_=kt_d[c * P:(c + 1) * P, :])
            vt_in = ins_pool.tile([P, DCA, m_pad], BF16, name="vt_in")
            for c in range(DCA):
                nc.sync.dma_start(out=vt_in[:, c, :], in_=vt_d[c * P:(c + 1) * P, :])

            # q^T [dq, n] and k^T [dk, m]
            for dqc in range(DGC):
                for i in range(NI5):
                    ps = psA.tile([P, 512], F32)
                    for c in range(DC):
                        nc.tensor.matmul(
                            ps,
                            lhsT=wq_sb[:, c, dqc * P:(dqc + 1) * P],
                            rhs=qt_in[:, c, i * 512:(i + 1) * 512],
                            start=(c == 0), stop=(c == DC - 1),
                        )
                    nc.vector.tensor_scalar_add(
                        qT[dqc][:, i * 512:(i + 1) * 512], ps, bq_sb[:, dqc:dqc + 1]
                    )
            for dkc in range(DGC):
                for j0 in range(0, m_pad, 512):
                    w = min(512, m_pad - j0)
                    ps = psA.tile([P, 512], F32)
                    for c in range(DC):
                        nc.tensor.matmul(
                            ps[:, :w],
                            lhsT=wk_sb[:, c, dkc * P:(dkc + 1) * P],
                            rhs=kt_in[:, c, j0:j0 + w],
                            start=(c == 0), stop=(c == DC - 1),
                        )
                    nc.vector.tensor_scalar_add(
                        kT[dkc][:, j0:j0 + w], ps[:, :w], bk_sb[:, dkc:dkc + 1]
                    )
            # v natural [j, dv] + ones column (softmax denominator)
            for j in range(NJ):
                ps = psA.tile([P, DG], F32)
                for c in range(DCA):
                    nc.tensor.matmul(
                        ps,
                        lhsT=vt_in[:, c, j * P:(j + 1) * P],
                        rhs=wv_sb[:, c, :],
                        start=(c == 0), stop=(c == DCA - 1),
                    )
                nc.vector.tensor_copy(
                    v_sb[j][:, :, 0:HD], ps.rearrange("p (h d) -> p h d", h=HL)
                )
                nc.vector.memset(v_sb[j][:, :, HD:HD + 1], 1.0)

        # exp(bias^T), resident bf16
        with ExitStack() as bctx:
            btp = bctx.enter_context(tc.tile_pool(name="btp", bufs=2))
            for j in range(NJ):
                bt = btp.tile([P, N], F32)
                nc.sync.dma_start(out=bt, in_=bt_d[j * P:(j + 1) * P, :])
                nc.scalar.activation(eb[j], bt, AF.Exp)

        # ---- stage B: attention ----
        psS = ctx.enter_context(tc.tile_pool(name="psS", bufs=3, space="PSUM"))
        psO = ctx.enter_context(tc.tile_pool(name="psO", bufs=2, space="PSUM"))
        etp = ctx.enter_context(tc.tile_pool(name="etp", bufs=4))
        ptp = ctx.enter_context(tc.tile_pool(name="ptp", bufs=4))
        rp = ctx.enter_context(tc.tile_pool(name="rp", bufs=2))

        for h in range(HL):
            hc, hh = divmod(h, 2)
            pr = slice(hh * HD, (hh + 1) * HD)
            for i in range(NI5):
                isl = slice(i * 512, (i + 1) * 512)
                po = psO.tile([HD + 1, 512], F32)
                for j in range(NJ):
                    ps = psS.tile([P, 512], F32)
                    nc.tensor.matmul(
                        ps,
                        lhsT=kT[hc][pr, j * P:(j + 1) * P],
                        rhs=qT[hc][pr, isl],
                        start=True, stop=True,
                    )
                    et = etp.tile([P, 512], BF16)
                    nc.scalar.activation(et, ps, AF.Exp, scale=SCALE)
                    pt = ptp.tile([P, 512], BF16)
                    nc.vector.tensor_mul(pt, et, eb[j][:, isl])
                    nc.tensor.matmul(
                        po, lhsT=v_sb[j][:, h, :], rhs=pt,
                        start=(j == 0), stop=(j == NJ - 1),
                    )
                r = rp.tile([1, 512], F32, name="r")
                nc.vector.reciprocal(r, po[HD:HD + 1, :])
                rb = rp.tile([HD, 512], F32, name="rb")
                nc.gpsimd.partition_broadcast(rb, r)
                nc.vector.tensor_mul(otn[(hc, i)][pr, :], po[0:HD, :], rb)

        # ---- stage C: output projection ----
        psF = ctx.enter_context(tc.tile_pool(name="psF", bufs=2, space="PSUM"))
        op = ctx.enter_context(tc.tile_pool(name="op", bufs=3))
        for i in range(N // P):
            i5, sub = divmod(i, 4)
            for oh in range(D // 512):
                pf = psF.tile([P, 512], F32)
                for hp in range(DGC):
                    nc.tensor.matmul(
                        pf,
                        lhsT=otn[(hp, i5)][:, sub * P:(sub + 1) * P],
                        rhs=wo_sb[:, hp, oh * 512:(oh + 1) * 512],
                        start=(hp == 0), stop=(hp == DGC - 1),
                    )
                ot = op.tile([P, 512], F32)
                nc.vector.tensor_copy(ot, pf)
                nc.sync.dma_start(
                    out=out_d[i * P:(i + 1) * P, oh * 512:(oh + 1) * 512], in_=ot
                )

    nc.compile()
    return nc


def _get(m_pad: int):
    if m_pad not in _cache:
        _cache[m_pad] = _build(m_pad)
    return _cache[m_pad]


_last_m_pad = None


def _prepare_in_maps(inputs, m_pad=None):
    Q = np.asarray(inputs["Q"])
    K = np.asarray(inputs["K"])
    V = np.asarray(inputs["V"])
    attn_bias = np.asarray(inputs["attn_bias"])
    mask = np.asarray(inputs["mask"])
    Wq, Wk, Wv, Wo = (np.asarray(inputs[k], np.float32) for k in ("Wq", "Wk", "Wv", "Wo"))
    bq, bk, bv = (np.asarray(inputs[k], np.float32) for k in ("bq", "bk", "bv"))
    bf = ml_dtypes.bfloat16

    idx = [np.flatnonzero(~mask[b]) for b in range(B)]
    if m_pad is None:
        m_pad = max(256, ((max(len(ix) for ix in idx) + P - 1) // P) * P)

    in_maps = []
    for c in range(2 * B):
        b, hg = divmod(c, HG)
        ix = idx[b]
        m = len(ix)
        sl = slice(hg * DG, (hg + 1) * DG)

        kt = np.zeros((D, m_pad), bf)
        kt[:, :m] = K[b][ix].T
        vt = np.zeros((DCA * P, m_pad), bf)
        vt[:D, :m] = V[b][ix].T
        vt[D, :m] = 1.0
        bt = np.full((m_pad, N), NEG, np.float32)
        bt[:m, :] = attn_bias[b].T[ix]
        wvt = np.zeros((DCA * P, DG), bf)
        wvt[:D] = Wv[sl, :].T
        wvt[D] = bv[sl]

        in_maps.append({
            "qt": np.ascontiguousarray(Q[b].T).astype(bf),
            "kt": kt,
            "vt": vt,
            "bt": bt,
            "wqt": np.ascontiguousarray(Wq[sl, :].T).astype(bf),
            "wkt": np.ascontiguousarray(Wk[sl, :].T).astype(bf),
            "wvt": wvt,
            "wot": np.ascontiguousarray(Wo[:, sl].T).astype(bf),
            "bqv": np.ascontiguousarray(bq[sl].reshape(DGC, P).T),
            "bkv": np.ascontiguousarray(bk[sl].reshape(DGC, P).T),
        })
    return in_maps, m_pad


def kernel(Q, K, V, attn_bias, mask, Wq, bq, Wk, bk, Wv, bv, Wo, bo):
    global _last_m_pad
    inputs = dict(Q=Q, K=K, V=V, attn_bias=attn_bias, mask=mask,
                  Wq=Wq, bq=bq, Wk=Wk, bk=bk, Wv=Wv, bv=bv, Wo=Wo, bo=bo)
    in_maps, m_pad = _prepare_in_maps(inputs)
    _last_m_pad = m_pad
    nc = _get(m_pad)
    bo = np.asarray(bo, np.float32)

    res = run_bass_kernel_spmd(nc, in_maps, list(range(2 * B)))
    out = np.empty((B, N, D), np.float32)
    for b in range(B):
        out[b] = res.results[2 * b]["out"] + res.results[2 * b + 1]["out"] + bo
    kernel.last_result = res
    return out


# revision 10
# speedup vs baseline: 3.9553x; 3.9553x over previous
"""Multi-head attention (B=4, N=M=2048, D=1024, H=16) on 8 trn2 NeuronCores.

Sharding: core c = (batch b = c//2, head-group hg = c%2 of 8 heads).
Each core computes its 8 heads end-to-end (fc_Q/K/V column-sharded by head,
fc_O row-sharded); the fc_O all-reduce over the 2 cores of a batch is done
in the host-side gather (a single np add), along with the +bo bias.

Host-side prep (sharding/relayout only):
  - Q/K/V are pre-transposed (feature-major) so every matmul contracts over
    the partition dim without any on-chip transposes.
  - Key rows with mask=True contribute exactly 0 to softmax (exp(-inf)); they
    are compacted away host-side and the key dim padded to a multiple of 128
    with bias^T rows of -1e30 (exp -> 0). ~Halves all key-side work.
  - bias enters via exp(s*scale + b) = exp(s*scale) * exp(b): exp(bias^T) is
    computed once on-chip (ACT) and multiplied in on DVE at bf16 2x rate.
  - softmax denominator comes free from a ones-column appended to V; the
    1/denom normalization is folded into the PSUM->SBUF copyback multiply
    before the output projection.
"""

import os
from contextlib import ExitStack

import numpy as np
import ml_dtypes

import concourse.bass as bass
import concourse.tile as tile
from concourse import bacc, mybir
from concourse.bass_utils import run_bass_kernel_spmd

BF16 = mybir.dt.bfloat16
F32 = mybir.dt.float32
AF = mybir.ActivationFunctionType

B, N, M, D, H = 4, 2048, 2048, 1024, 16
HG = 2            # head-groups (cores per batch)
HL = H // HG      # heads per core
HD = D // H       # head dim
DG = HL * HD      # per-core projection width (512)
P = 128
DC = D // P       # D chunks (8)
DCA = DC + 1      # aug chunks for V (bias row)
DGC = DG // P     # head-dim chunks per core (4)
NI5 = N // 512    # query tiles of 512
NEG = -1e30
SCALE = 1.0 / float(np.sqrt(HD))

_cache: dict[int, object] = {}


def _build(m_pad: int):
    NJ = m_pad // P
    nc = bacc.Bacc("TRN2", target_bir_lowering=False, debug=False, num_devices=8)

    qt_d = nc.dram_tensor("qt", [D, N], BF16, kind="ExternalInput").ap()
    kt_d = nc.dram_tensor("kt", [D, m_pad], BF16, kind="ExternalInput").ap()
    vt_d = nc.dram_tensor("vt", [DCA * P, m_pad], BF16, kind="ExternalInput").ap()
    bt_d = nc.dram_tensor("bt", [m_pad, N], F32, kind="ExternalInput").ap()
    wqt_d = nc.dram_tensor("wqt", [D, DG], BF16, kind="ExternalInput").ap()
    wkt_d = nc.dram_tensor("wkt", [D, DG], BF16, kind="ExternalInput").ap()
    wvt_d = nc.dram_tensor("wvt", [DCA * P, DG], BF16, kind="ExternalInput").ap()
    wot_d = nc.dram_tensor("wot", [DG, D], BF16, kind="ExternalInput").ap()
    bq_d = nc.dram_tensor("bqv", [P, DGC], F32, kind="ExternalInput").ap()
    bk_d = nc.dram_tensor("bkv", [P, DGC], F32, kind="ExternalInput").ap()
    out_d = nc.dram_tensor("out", [N, D], F32, kind="ExternalOutput").ap()

    with tile.TileContext(nc) as tc, ExitStack() as ctx:
        singles = ctx.enter_context(tc.tile_pool(name="singles", bufs=1))

        # resident weights
        wq_sb = singles.tile([P, DC, DG], BF16, name="wq")
        nc.sync.dma_start(out=wq_sb, in_=wqt_d.rearrange("(c p) g -> p c g", p=P))
        wk_sb = singles.tile([P, DC, DG], BF16, name="wk")
        nc.sync.dma_start(out=wk_sb, in_=wkt_d.rearrange("(c p) g -> p c g", p=P))
        wo_sb = singles.tile([P, DGC, D], BF16, name="wo")
        nc.sync.dma_start(out=wo_sb, in_=wot_d.rearrange("(c p) o -> p c o", p=P))
        bq_sb = singles.tile([P, DGC], F32, name="bq")
        nc.sync.dma_start(out=bq_sb, in_=bq_d)
        bk_sb = singles.tile([P, DGC], F32, name="bk")
        nc.sync.dma_start(out=bk_sb, in_=bk_d)

        # persistent activations
        qT = [singles.tile([P, N], BF16, name=f"qT{c}") for c in range(DGC)]
        kT = [singles.tile([P, m_pad], BF16, name=f"kT{c}") for c in range(DGC)]
        v_sb = [singles.tile([P, HL, HD + 1], BF16, name=f"v{j}") for j in range(NJ)]
        eb = [singles.tile([P, N], BF16, name=f"eb{j}") for j in range(NJ)]
        ins_pool = ctx.enter_context(tc.tile_pool(name="ins", bufs=1))
        kt_in = ins_pool.tile([P, DC, m_pad], BF16, name="kt_in")
        for c in range(DC):
            nc.sync.dma_start(out=kt_in[:, c, :], in_=kt_d[c * P:(c + 1) * P, :])
        qt_in = ins_pool.tile([P, DC, N], BF16, name="qt_in")
        for c in range(DC):
            nc.sync.dma_start(out=qt_in[:, c, :], in_=qt_d[c * P:(c + 1) * P, :])

        psA = ctx.enter_context(tc.tile_pool(name="psA", bufs=2, space="PSUM"))

        # early phase: exp(bias^T) on ACT while PE projects v
        with ExitStack() as ectx:
            vtp = ectx.enter_context(tc.tile_pool(name="vtp", bufs=1))
            btp = ectx.enter_context(tc.tile_pool(name="btp", bufs=2))
            wv_sb = vtp.tile([P, DCA, DG], BF16, name="wv")
            nc.sync.dma_start(out=wv_sb, in_=wvt_d.rearrange("(c p) g -> p c g", p=P))
            vt_in = vtp.tile([P, DCA, m_pad], BF16, name="vt_in")
            for c in range(DCA):
                nc.sync.dma_start(out=vt_in[:, c, :], in_=vt_d[c * P:(c + 1) * P, :])
            for j in range(NJ):
                bt = btp.tile([P, N], F32)
                nc.sync.dma_start(out=bt, in_=bt_d[j * P:(j + 1) * P, :])
                nc.scalar.activation(eb[j], bt, AF.Exp)
            # v natural [j, dv] + ones column (softmax denominator)
            for j in range(NJ):
                ps = psA.tile([P, DG], F32, name="psa")
                for c in range(DCA):
                    nc.tensor.matmul(
                        ps,
                        lhsT=vt_in[:, c, j * P:(j + 1) * P],
                        rhs=wv_sb[:, c, :],
                        start=(c == 0), stop=(c == DCA - 1),
                    )
                nc.vector.tensor_copy(
                    v_sb[j][:, :, 0:HD], ps.rearrange("p (h d) -> p h d", h=HL)
                )
                nc.vector.memset(v_sb[j][:, :, HD:HD + 1], 1.0)

        # stage B pools
        otnp = ctx.enter_context(tc.tile_pool(name="otnp", bufs=1))
        otn = {}
        for hp in range(HL // 2):
            for i in range(NI5):
                otn[(hp, i)] = otnp.tile([P, 512], BF16, name=f"otn{hp}_{i}")
        psS = ctx.enter_context(tc.tile_pool(name="psS", bufs=2, space="PSUM"))
        psO = ctx.enter_context(tc.tile_pool(name="psO", bufs=2, space="PSUM"))
        etp = ctx.enter_context(tc.tile_pool(name="etp", bufs=3))
        ptp = ctx.enter_context(tc.tile_pool(name="ptp", bufs=3))
        rp = ctx.enter_context(tc.tile_pool(name="rp", bufs=4))
        op = ctx.enter_context(tc.tile_pool(name="op", bufs=3))
        ncopy = 0

        for hp in range(HL // 2):
            h0, h1 = 2 * hp, 2 * hp + 1
            # project k^T and q^T for this head-pair's chunk
            for j0 in range(0, m_pad, 512):
                w = min(512, m_pad - j0)
                ps = psA.tile([P, 512], F32, name="psa")
                for c in range(DC):
                    nc.tensor.matmul(
                        ps[:, :w],
                        lhsT=wk_sb[:, c, hp * P:(hp + 1) * P],
                        rhs=kt_in[:, c, j0:j0 + w],
                        start=(c == 0), stop=(c == DC - 1),
                    )
                nc.vector.tensor_scalar_add(
                    kT[hp][:, j0:j0 + w], ps[:, :w], bk_sb[:, hp:hp + 1]
                )
            for i in range(NI5):
                ps = psA.tile([P, 512], F32, name="psa")
                for c in range(DC):
                    nc.tensor.matmul(
                        ps,
                        lhsT=wq_sb[:, c, hp * P:(hp + 1) * P],
                        rhs=qt_in[:, c, i * 512:(i + 1) * 512],
                        start=(c == 0), stop=(c == DC - 1),
                    )
                nc.vector.tensor_scalar_add(
                    qT[hp][:, i * 512:(i + 1) * 512], ps, bq_sb[:, hp:hp + 1]
                )
            # attention for heads (h0, h1): packed row-tiled score matmuls
            for i in range(NI5):
                isl = slice(i * 512, (i + 1) * 512)
                po0 = psO.tile([HD + 1, 512], F32, name="po")
                po1 = psO.tile([HD + 1, 512], F32, name="po")
                for j in range(NJ):
                    ps = psS.tile([P, 1024], F32)
                    nc.tensor.matmul(
                        ps[:, 0:512],
                        lhsT=kT[hp][0:HD, j * P:(j + 1) * P],
                        rhs=qT[hp][0:HD, isl],
                        start=True, stop=True, tile_position=(0, 0),
                    )
                    nc.tensor.matmul(
                        ps[:, 512:1024],
                        lhsT=kT[hp][HD:P, j * P:(j + 1) * P],
                        rhs=qT[hp][HD:P, isl],
                        start=True, stop=True, tile_position=(64, 0),
                    )
                    et = etp.tile([P, 1024], BF16)
                    nc.scalar.activation(et, ps, AF.Exp, scale=SCALE)
                    pt = ptp.tile([P, 1024], BF16)
                    ebs = eb[j][:, isl]
                    eb2 = bass.AP(
                        tensor=ebs.tensor, offset=ebs.offset,
                        ap=[ebs.ap[0], [0, 2], ebs.ap[1]],
                    )
                    nc.vector.tensor_mul(
                        pt.rearrange("p (r c) -> p r c", r=2),
                        et.rearrange("p (r c) -> p r c", r=2),
                        eb2,
                    )
                    nc.tensor.matmul(
                        po0, lhsT=v_sb[j][:, h0, :], rhs=pt[:, 0:512],
                        start=(j == 0), stop=(j == NJ - 1),
                    )
                    nc.tensor.matmul(
                        po1, lhsT=v_sb[j][:, h1, :], rhs=pt[:, 512:1024],
                        start=(j == 0), stop=(j == NJ - 1),
                    )
                for hh, po in ((0, po0), (1, po1)):
                    r = rp.tile([1, 512], F32, name="r")
                    nc.vector.reciprocal(r, po[HD:HD + 1, :])
                    rb = rp.tile([HD, 512], F32, name="rb")
                    nc.gpsimd.partition_broadcast(rb, r)
                    nc.vector.tensor_mul(
                        otn[(hp, i)][hh * HD:(hh + 1) * HD, :], po[0:HD, :], rb
                    )

                # stage C: after the last head-pair, this i-block is complete
                if hp == HL // 2 - 1:
                    for sub in range(4):
                        ii = i * 4 + sub
                        for oh in range(D // 512):
                            pf = psA.tile([P, 512], F32, name="psa")
                            for hpp in range(DGC):
                                nc.tensor.matmul(
                                    pf,
                                    lhsT=otn[(hpp, i)][:, sub * P:(sub + 1) * P],
                                    rhs=wo_sb[:, hpp, oh * 512:(oh + 1) * 512],
                                    start=(hpp == 0), stop=(hpp == DGC - 1),
                                )
                            ot = op.tile([P, 512], F32)
                            if ncopy % 2 == 0 or os.environ.get("NO_ACT_COPY"):
                                nc.vector.tensor_copy(ot, pf)
                            else:
                                nc.scalar.activation(ot, pf, AF.Copy)
                            ncopy += 1
                            nc.sync.dma_start(
                                out=out_d[ii * P:(ii + 1) * P, oh * 512:(oh + 1) * 512],
                                in_=ot,
                            )

    nc.compile()
    return nc


def _get(m_pad: int):
    if m_pad not in _cache:
        _cache[m_pad] = _build(m_pad)
    return _cache[m_pad]


_last_m_pad = None


def _prepare_in_maps(inputs, m_pad=None):
    Q = np.asarray(inputs["Q"])
    K = np.asarray(inputs["K"])
    V = np.asarray(inputs["V"])
    attn_bias = np.asarray(inputs["attn_bias"])
    mask = np.asarray(inputs["mask"])
    Wq, Wk, Wv, Wo = (np.asarray(inputs[k], np.float32) for k in ("Wq", "Wk", "Wv", "Wo"))
    bq, bk, bv = (np.asarray(inputs[k], np.float32) for k in ("bq", "bk", "bv"))
    bf = ml_dtypes.bfloat16

    idx = [np.flatnonzero(~mask[b]) for b in range(B)]
    if m_pad is None:
        m_pad = max(256, ((max(len(ix) for ix in idx) + P - 1) // P) * P)

    in_maps = []
    for c in range(2 * B):
        b, hg = divmod(c, HG)
        ix = idx[b]
        m = len(ix)
        sl = slice(hg * DG, (hg + 1) * DG)

        kt = np.zeros((D, m_pad), bf)
        kt[:, :m] = K[b][ix].T
        vt = np.zeros((DCA * P, m_pad), bf)
        vt[:D, :m] = V[b][ix].T
        vt[D, :m] = 1.0
        bt = np.full((m_pad, N), NEG, np.float32)
        bt[:m, :] = attn_bias[b].T[ix]
        wvt = np.zeros((DCA * P, DG), bf)
        wvt[:D] = Wv[sl, :].T
        wvt[D] = bv[sl]

        in_maps.append({
            "qt": np.ascontiguousarray(Q[b].T).astype(bf),
            "kt": kt,
            "vt": vt,
            "bt": bt,
            "wqt": np.ascontiguousarray(Wq[sl, :].T).astype(bf),
            "wkt": np.ascontiguousarray(Wk[sl, :].T).astype(bf),
            "wvt": wvt,
            "wot": np.ascontiguousarray(Wo[:, sl].T).astype(bf),
            "bqv": np.ascontiguousarray(bq[sl].reshape(DGC, P).T),
            "bkv": np.ascontiguousarray(bk[sl].reshape(DGC, P).T),
        })
    return in_maps, m_pad


def kernel(Q, K, V, attn_bias, mask, Wq, bq, Wk, bk, Wv, bv, Wo, bo):
    global _last_m_pad
    inputs = dict(Q=Q, K=K, V=V, attn_bias=attn_bias, mask=mask,
                  Wq=Wq, bq=bq, Wk=Wk, bk=bk, Wv=Wv, bv=bv, Wo=Wo, bo=bo)
    in_maps, m_pad = _prepare_in_maps(inputs)
    _last_m_pad = m_pad
    nc = _get(m_pad)
    bo = np.asarray(bo, np.float32)

    res = run_bass_kernel_spmd(nc, in_maps, list(range(2 * B)))
    out = np.empty((B, N, D), np.float32)
    for b in range(B):
        out[b] = res.results[2 * b]["out"] + res.results[2 * b + 1]["out"] + bo
    kernel.last_result = res
    return out
